# revision 16
# baseline (speedup 1.0000x reference)
"""ErnieLayout self-attention on 8 Trainium2 NeuronCores (Bass/Tile).

Problem shapes (hardcoded): B=4, S=1024, H=768, NH=12, HD=64.
Sharding: core c -> (batch b = c//2, head-half hh = c%2, i.e. 6 heads).
Each core computes attention for its 6 heads of one batch element and
writes the [S, 384] column slice of that batch's output.

Per-core algorithm (v2, mixed precision):
  setup:  X^T and W^T slices via PE transposes (fp32);
          Q^T = (Wq_s @ X^T + bq)/8, K^T = Wk_s @ X^T + bk  (fp32r matmuls)
          V   = X @ Wv_s^T + bv, ones column appended, stored bf16
  per (head, ktile, q-chunk):
          psum[k=128, q=512] = K^T.T @ Q^T             (fp32r, 1 cyc/row)
          psum += rel12[q,k-tile]^T via matmul(lhsT=rel12_bf16, rhs=I_bf16)
          pT = exp(psum + maskbias[k]) -> bf16         (ACT per-partition bias)
  per (head, qt): ctx[q,65] += pT.T @ V_aug[kt] over kt (bf16; col 64 = denom)
  finalize: out[q, h*64:..] = ctx[:, :64] * (1/ctx[:, 64])  (fp32)

rel12 = rel_pos + rel_2d_pos is one DVE pass (fp32 in, bf16 out).
Masked keys get bias FLT_MIN so exp underflows to exactly 0 (matches the
reference's FLT_MIN replacement; no row-max needed, scores are O(10)).
Precision: rel12/P/V in bf16 and qk in fp32r -> final rel err ~1e-4.
"""

import os
import sys

import numpy as np

for _p in ("/opt/trn_rl_repo",):
    if _p not in sys.path and os.path.isdir(_p):
        sys.path.append(_p)

import concourse.bass as bass
import concourse.mybir as mybir
import concourse.tile as tile
from concourse import bacc
from concourse.bass_utils import run_bass_kernel_spmd
from concourse.masks import make_identity

F32 = mybir.dt.float32
F32R = mybir.dt.float32r
BF16 = mybir.dt.bfloat16
F16 = mybir.dt.float16
I32 = mybir.dt.int32
AF = mybir.ActivationFunctionType
NEG = float(np.finfo(np.float32).min)

P = 128
S = 1024
NH = 6        # heads per core
HD = 64
HIN = 768     # model dim (contraction for projections)
HOUT = NH * HD  # 384, per-core projection width
KT = S // P   # 8 key tiles
QT = S // P   # 8 query tiles
VW = HD + 1   # 65: V columns + ones column


def _r(ap):
    """bitcast fp32 AP to fp32r for fast single-pass matmul"""
    return ap.bitcast(F32R)


def _build_kernel_body(tc, aps):
    import contextlib

    nc = tc.nc
    x_ap = aps["x"]
    mask_ap = aps["mask"]
    rel1_ap = aps["rel1"]
    rel2_ap = aps["rel2"]
    out_ap = aps["out"]

    with contextlib.ExitStack() as ctx:
        const = ctx.enter_context(tc.tile_pool(name="const", bufs=1))

        ident = const.tile([P, P], F32)
        make_identity(nc, ident)
        ident_bf = const.tile([P, P], F16)
        nc.vector.tensor_copy(ident_bf[:], ident[:])

        # mask bias: maskb[p, kt] = FLT_MIN if key (kt*128+p) masked else 0
        mask_i = const.tile([P, KT], I32)
        nc.sync.dma_start(mask_i[:], mask_ap.rearrange("(a p) -> p a", p=P))
        maskb = const.tile([P, KT], F32)
        nc.vector.tensor_copy(maskb[:], mask_i[:])
        nc.vector.tensor_scalar_mul(maskb[:], maskb[:], NEG)

        # biases: [384] -> [128, 3] (per-partition columns per d-tile)
        bias_sb = {}
        for wname in ("q", "k"):
            bt = const.tile([P, 3], F32, tag=f"b{wname}")
            nc.sync.dma_start(bt[:], aps[f"b{wname}"].rearrange("(a p) -> p a", p=P))
            if wname == "q":
                nc.vector.tensor_scalar_mul(bt[:], bt[:], 0.125)
            bias_sb[wname] = bt
        bv_row = const.tile([1, HOUT], F32)
        nc.sync.dma_start(bv_row[:], aps["bv"][None, :])
        ones_col = const.tile([1, P], F32)
        nc.vector.memset(ones_col[:], 1.0)

        # long-lived tensors
        qt_pool = ctx.enter_context(tc.tile_pool(name="qT", bufs=3))
        kt_pool = ctx.enter_context(tc.tile_pool(name="kT", bufs=3))
        v_pool = ctx.enter_context(tc.tile_pool(name="v", bufs=8))

        qT = [qt_pool.tile([P, S], F32R, tag="qT", name=f"qT{i}") for i in range(3)]
        kT = [kt_pool.tile([P, S], F32R, tag="kT", name=f"kT{i}") for i in range(3)]
        v_tiles = [
            v_pool.tile([P, NH, VW], F16, tag="v", name=f"v{i}") for i in range(8)
        ]

        # rel stream pools first: their SBUF is disjoint from phase-1 pools,
        # so rel DMA + DVE adds overlap phase 1 from t=0
        r1_pool = ctx.enter_context(tc.tile_pool(name="r1", bufs=2))
        r2_pool = ctx.enter_context(tc.tile_pool(name="r2", bufs=2))
        rbf_pool = ctx.enter_context(tc.tile_pool(name="rbf", bufs=10))

        # ---------------- phase 1: load, transpose, project ----------------
        with contextlib.ExitStack() as ph1:
            xload = ph1.enter_context(tc.tile_pool(name="xload", bufs=8))
            wload = ph1.enter_context(tc.tile_pool(name="wload", bufs=3))
            xt_pool = ph1.enter_context(tc.tile_pool(name="xT", bufs=6))
            wt_pool = ph1.enter_context(tc.tile_pool(name="wT", bufs=18))
            psum1 = ph1.enter_context(tc.tile_pool(name="psum1", bufs=2, space="PSUM"))
            psum1b = ph1.enter_context(
                tc.tile_pool(name="psum1b", bufs=2, space="PSUM")
            )

            # X tiles [128, 768]
            x_tiles = []
            for t in range(8):
                xt_ = xload.tile([P, HIN], F32, tag="x")
                nc.sync.dma_start(xt_[:], x_ap[t * P:(t + 1) * P, :])
                x_tiles.append(xt_)

            # X^T: 6 tiles [128, 1024] (h-chunk on partitions)
            xT = []
            for hc in range(6):
                pt = psum1.tile([P, S], F32, tag="xtp")  # 2 banks
                for t in range(8):
                    nc.tensor.transpose(
                        pt[:, t * P:(t + 1) * P],
                        x_tiles[t][:, hc * P:(hc + 1) * P],
                        ident[:],
                    )
                xt_t = xt_pool.tile([P, S], F32R, tag="xT")
                nc.scalar.copy(xt_t[:], pt[:])
                xT.append(xt_t)

            # W^T slices: wT[(w, hc)] = [128, 384]
            wT = {}
            for wname in ("q", "k", "v"):
                w_ap = aps[f"w{wname}"]
                wtiles = []
                for d in range(3):
                    wt_ = wload.tile([P, HIN], F32, tag="wload")
                    nc.sync.dma_start(wt_[:], w_ap[d * P:(d + 1) * P, :])
                    wtiles.append(wt_)
                for hc in range(6):
                    pw = psum1b.tile([P, 512], F32, tag="ps1b", name="pw")[:, :HOUT]
                    for d in range(3):
                        nc.tensor.transpose(
                            pw[:, d * P:(d + 1) * P],
                            wtiles[d][:, hc * P:(hc + 1) * P],
                            ident[:],
                        )
                    wt_t = wt_pool.tile([P, HOUT], F32R, tag="wT")
                    nc.scalar.copy(wt_t[:], pw[:])
                    wT[(wname, hc)] = wt_t

            # Q^T, K^T projections: out [d-tile 128, t-chunk 512], fp32r
            for wname, dest, scale in (("q", qT, 0.125), ("k", kT, 1.0)):
                for d in range(3):
                    for tch in range(2):
                        pp = psum1b.tile([P, 512], F32, tag="ps1b")
                        for hc in range(6):
                            nc.tensor.matmul(
                                pp[:],
                                wT[(wname, hc)][:, d * P:(d + 1) * P],
                                xT[hc][:, tch * 512:(tch + 1) * 512],
                                start=(hc == 0),
                                stop=(hc == 5),
                            )
                        nc.scalar.activation(
                            dest[d][:, tch * 512:(tch + 1) * 512],
                            pp[:],
                            AF.Identity,
                            bias=bias_sb[wname][:, d:d + 1],
                            scale=scale,
                        )

            # V projection: out [t-tile 128, 384] + ones column, bf16 out
            for t in range(8):
                pv = psum1b.tile([P, 512], F32, tag="ps1b", name="pv")[:, :HOUT]
                for hc in range(6):
                    nc.tensor.matmul(
                        pv[:],
                        xT[hc][:, t * P:(t + 1) * P],
                        wT[("v", hc)][:],
                        start=(hc == 0),
                        stop=False,
                    )
                nc.tensor.matmul(
                    pv[:], ones_col[:1, :], bv_row[:1, :], start=False, stop=True
                )
                nc.vector.memset(v_tiles[t][:], 1.0)
                nc.scalar.copy(
                    v_tiles[t][:, :, 0:HD],
                    pv[:].rearrange("p (h d) -> p h d", d=HD),
                )

        # ---------------- phase 2: attention per head ----------------
        # rel strips stream in quarter-head granularity [128, 2, 1024] fp32,
        # summed into bf16 quarters used as transposing-matmul weights.
        out_pool = ctx.enter_context(tc.tile_pool(name="outst", bufs=8))
        out_stage = [
            out_pool.tile([P, HOUT], F32, tag="outst", name=f"outst{i}")
            for i in range(8)
        ]
        pt_pool = ctx.enter_context(tc.tile_pool(name="pT", bufs=18))
        fin_pool = ctx.enter_context(tc.tile_pool(name="fin", bufs=4))
        spsum = ctx.enter_context(tc.tile_pool(name="spsum", bufs=4, space="PSUM"))
        cpsum = ctx.enter_context(tc.tile_pool(name="cpsum", bufs=4, space="PSUM"))

        # Heads are processed in pairs (2*hp, 2*hp+1) = one d-tile: their
        # K=64 qk matmuls sit at lhsT base partitions 0/64 and row-tile
        # into the PE array concurrently; the denser PE stream also keeps
        # the HAM clock-gate warm.
        for hp in range(NH // 2):
            heads = (2 * hp, 2 * hp + 1)
            # rel12 = rel1 + rel2 -> fp16, four quarter tiles per head
            quarters = {}
            for h01, h in enumerate(heads):
                for qq in range(4):
                    r1 = r1_pool.tile([P, 2, S], F32, tag="r1")
                    nc.sync.dma_start(
                        r1[:],
                        rel1_ap[h].rearrange("(qt p) k -> p qt k", p=P)[
                            :, qq * 2:(qq + 1) * 2, :
                        ],
                    )
                    r2 = r2_pool.tile([P, 2, S], F32, tag="r2")
                    nc.sync.dma_start(
                        r2[:],
                        rel2_ap[h].rearrange("(qt p) k -> p qt k", p=P)[
                            :, qq * 2:(qq + 1) * 2, :
                        ],
                    )
                    rb = rbf_pool.tile(
                        [P, 2, S], F16, tag="rbf", name=f"rbf{h}_{qq}"
                    )
                    nc.vector.tensor_add(rb[:], r1[:], r2[:])
                    quarters[(h01, qq)] = rb

            ctx_ps = [
                cpsum.tile([P, 4 * VW], F32, tag="ctx", name=f"ctx{hp}_{i}")
                for i in range(4)
            ]
            pT_strips = {}
            for kt in range(KT):
                for h01 in range(2):
                    pT_strips[(h01, kt)] = pt_pool.tile(
                        [P, S], F16, tag="pT", name=f"pT{hp}_{h01}_{kt}"
                    )
                # qk^T for both heads back-to-back: row-tiled concurrent MMs
                ps_tiles = {}
                for qch in range(2):
                    for h01 in range(2):
                        d0 = h01 * HD
                        ps = spsum.tile([P, 512], F32, tag="sT",
                                        name=f"ps{hp}_{kt}_{qch}_{h01}")
                        ps_tiles[(h01, qch)] = ps
                        nc.tensor.matmul(
                            ps[:],
                            kT[hp][d0:d0 + HD, kt * P:(kt + 1) * P],
                            qT[hp][d0:d0 + HD, qch * 512:(qch + 1) * 512],
                            start=True,
                            stop=False,
                        )
                # += rel12^T (transposing adds via fp16 identity rhs)
                for qch in range(2):
                    for h01 in range(2):
                        ps = ps_tiles[(h01, qch)]
                        for j in range(4):
                            qt = qch * 4 + j
                            nc.tensor.matmul(
                                ps[:, j * P:(j + 1) * P],
                                quarters[(h01, qt // 2)][
                                    :, qt % 2, kt * P:(kt + 1) * P
                                ],
                                ident_bf[:],
                                start=False,
                                stop=(j == 3),
                            )
                        # exp(scores + mask bias) -> fp16 probs
                        nc.scalar.activation(
                            pT_strips[(h01, kt)][:, qch * 512:(qch + 1) * 512],
                            ps[:],
                            AF.Exp,
                            bias=maskb[:, kt:kt + 1],
                            scale=1.0,
                        )

            # PV + denominator: per qt slot, contiguous accumulation group
            # (one open group per 2KB psum bank at a time)
            for h01, h in enumerate(heads):
                for qt in range(QT):
                    cp = ctx_ps[h01 * 2 + qt // 4]
                    sl = (qt % 4) * VW
                    for kt in range(KT):
                        nc.tensor.matmul(
                            cp[:, sl:sl + VW],
                            pT_strips[(h01, kt)][:, qt * P:(qt + 1) * P],
                            v_tiles[kt][:, h, :],
                            start=(kt == 0),
                            stop=(kt == KT - 1),
                        )

            # finalize: divide by denominator
            for h01, h in enumerate(heads):
                for qt in range(QT):
                    cp = ctx_ps[h01 * 2 + qt // 4]
                    sl = (qt % 4) * VW
                    rc = fin_pool.tile([P, 1], F32, tag="recip")
                    nc.vector.reciprocal(rc[:], cp[:, sl + HD:sl + HD + 1])
                    nc.vector.tensor_scalar_mul(
                        out_stage[qt][:, h * HD:(h + 1) * HD],
                        cp[:, sl:sl + HD],
                        rc[:],
                    )

        for qt in range(QT):
            nc.sync.dma_start(out_ap[qt * P:(qt + 1) * P, :], out_stage[qt][:])


def build_program():
    """Build and compile the per-core Bass program. Returns nc."""
    nc = bacc.Bacc(
        "TRN2",
        target_bir_lowering=False,
        debug=False,
        num_devices=8,
    )
    aps = {
        "x": nc.dram_tensor("x", [S, HIN], F32, kind="ExternalInput").ap(),
        "mask": nc.dram_tensor("mask", [S], I32, kind="ExternalInput").ap(),
        "rel1": nc.dram_tensor("rel1", [NH, S, S], F32, kind="ExternalInput").ap(),
        "rel2": nc.dram_tensor("rel2", [NH, S, S], F32, kind="ExternalInput").ap(),
        "wq": nc.dram_tensor("wq", [HOUT, HIN], F32, kind="ExternalInput").ap(),
        "wk": nc.dram_tensor("wk", [HOUT, HIN], F32, kind="ExternalInput").ap(),
        "wv": nc.dram_tensor("wv", [HOUT, HIN], F32, kind="ExternalInput").ap(),
        "bq": nc.dram_tensor("bq", [HOUT], F32, kind="ExternalInput").ap(),
        "bk": nc.dram_tensor("bk", [HOUT], F32, kind="ExternalInput").ap(),
        "bv": nc.dram_tensor("bv", [HOUT], F32, kind="ExternalInput").ap(),
        "out": nc.dram_tensor("out", [S, HOUT], F32, kind="ExternalOutput").ap(),
    }
    with tile.TileContext(nc) as tc:
        _build_kernel_body(tc, aps)
    nc.compile()
    return nc


def make_in_maps(inputs):
    """Slice full inputs into the 8 per-core input maps."""
    hs = np.ascontiguousarray(np.asarray(inputs["hidden_states"], np.float32))
    am = np.asarray(inputs["attention_mask"]).astype(np.int32)
    rel1 = np.asarray(inputs["rel_pos"], np.float32)
    rel2 = np.asarray(inputs["rel_2d_pos"], np.float32)
    ws = {k: np.asarray(inputs["W" + k[-1]], np.float32) for k in ("wq", "wk", "wv")}
    bs = {k: np.asarray(inputs["b" + k[-1]], np.float32) for k in ("bq", "bk", "bv")}

    in_maps = []
    for c in range(8):
        b, hh = divmod(c, 2)
        hsl = slice(hh * NH, (hh + 1) * NH)
        csl = slice(hh * HOUT, (hh + 1) * HOUT)
        m = {
            "x": np.ascontiguousarray(hs[b]),
            "mask": np.ascontiguousarray(am[b, 0, 0]),
            "rel1": np.ascontiguousarray(rel1[b, hsl]),
            "rel2": np.ascontiguousarray(rel2[b, hsl]),
        }
        for k in ("wq", "wk", "wv"):
            m[k] = np.ascontiguousarray(ws[k][csl])
        for k in ("bq", "bk", "bv"):
            m[k] = np.ascontiguousarray(bs[k][csl])
        in_maps.append(m)
    return in_maps


def gather_output(results):
    out = np.empty((4, S, HIN), np.float32)
    for c in range(8):
        b, hh = divmod(c, 2)
        out[b, :, hh * HOUT:(hh + 1) * HOUT] = results[c]["out"]
    return out


_NC_CACHE = []


def kernel(**inputs):
    if not _NC_CACHE:
        _NC_CACHE.append(build_program())
    nc = _NC_CACHE[0]
    in_maps = make_in_maps(inputs)
    res = run_bass_kernel_spmd(nc, in_maps, list(range(8)))
    return gather_output(res.results)


# revision 19
# speedup vs baseline: 1.1307x; 1.1307x over previous
"""ErnieLayout self-attention on 8 Trainium2 NeuronCores (Bass/Tile).

Problem shapes (hardcoded): B=4, S=1024, H=768, NH=12, HD=64.
Sharding: core c -> (batch b = c//2, head-half hh = c%2, i.e. 6 heads).
Each core computes attention for its 6 heads of one batch element and
writes the [S, 384] column slice of that batch's output.

Per-core algorithm (v3, mixed precision):
  setup:  X and W cast to fp16 (DVE), transposed on the PE (fp16 path);
          Q^T = (Wq_s @ X^T + bq)/8, K^T = Wk_s @ X^T + bk   (fp16 matmuls,
          fp32 PSUM accumulate, fp32r output tiles)
          V = X @ Wv_s^T (+ bv via DVE broadcast add), stored fp16 with a
          ones column appended (col 64 -> softmax denominator for free)
  per (head, ktile, q-chunk):
          psum[k=128, q=512] = K^T.T @ Q^T              (fp32r, 1 cyc/row)
          psum += rel12[q,ktile]^T via matmul(lhsT=rel12_f16, rhs=I_f16)
          pT = exp(psum + maskbias[k]) -> fp16          (ACT per-partition bias)
  per (head, qt): ctx[q,65] += pT.T @ V_aug[kt] over kt (fp16 matmuls)
  finalize: out[q, h*64:..] = ctx[:, :64] * (1/ctx[:, 64])  (fp32)

rel12 = rel_pos + rel_2d_pos is one DVE pass (fp32 in, fp16 out), streamed
in quarter-head strips so DMA prefetch runs continuously from t=0.
Masked keys get bias FLT_MIN so exp underflows to exactly 0 (matches the
reference's FLT_MIN replacement; no row-max needed, scores are O(10)).
Precision: fp16 carries >=10 mantissa bits -> final rel err ~1e-4..1e-3.
"""

import os
import sys

import numpy as np

for _p in ("/opt/trn_rl_repo",):
    if _p not in sys.path and os.path.isdir(_p):
        sys.path.append(_p)

import concourse.bass as bass
import concourse.mybir as mybir
import concourse.tile as tile
from concourse import bacc
from concourse.bass_utils import run_bass_kernel_spmd
from concourse.masks import make_identity

F32 = mybir.dt.float32
F32R = mybir.dt.float32r
F16 = mybir.dt.float16
I32 = mybir.dt.int32
AF = mybir.ActivationFunctionType
NEG = float(np.finfo(np.float32).min)

P = 128
S = 1024
NH = 6        # heads per core
HD = 64
HIN = 768     # model dim (contraction for projections)
HOUT = NH * HD  # 384, per-core projection width
KT = S // P   # 8 key tiles
QT = S // P   # 8 query tiles
VW = HD + 1   # 65: V columns + ones column


def _build_kernel_body(tc, aps):
    import contextlib

    nc = tc.nc
    x_ap = aps["x"]
    mask_ap = aps["mask"]
    rel1_ap = aps["rel1"]
    rel2_ap = aps["rel2"]
    out_ap = aps["out"]

    with contextlib.ExitStack() as ctx:
        const = ctx.enter_context(tc.tile_pool(name="const", bufs=1))

        ident = const.tile([P, P], F16)
        make_identity(nc, ident)

        # mask bias: maskb[p, kt] = FLT_MIN if key (kt*128+p) masked else 0
        mask_i = const.tile([P, KT], I32)
        nc.sync.dma_start(mask_i[:], mask_ap.rearrange("(a p) -> p a", p=P))
        maskb = const.tile([P, KT], F32)
        nc.vector.tensor_copy(maskb[:], mask_i[:])
        nc.vector.tensor_scalar_mul(maskb[:], maskb[:], NEG)

        # biases: [384] -> [128, 3] (per-partition columns per d-tile)
        bias_sb = {}
        for wname in ("q", "k"):
            bt = const.tile([P, 3], F32, tag=f"b{wname}")
            nc.sync.dma_start(bt[:], aps[f"b{wname}"].rearrange("(a p) -> p a", p=P))
            if wname == "q":
                nc.vector.tensor_scalar_mul(bt[:], bt[:], 0.125)
            bias_sb[wname] = bt
        bv_bc = const.tile([P, NH, HD], F32)
        nc.sync.dma_start(
            bv_bc[:],
            aps["bv"].rearrange("(h d) -> h d", d=HD)[None].to_broadcast((P, NH, HD)),
        )

        # long-lived tensors
        qt_pool = ctx.enter_context(tc.tile_pool(name="qT", bufs=3))
        kt_pool = ctx.enter_context(tc.tile_pool(name="kT", bufs=3))
        v_pool = ctx.enter_context(tc.tile_pool(name="v", bufs=8))

        qT = [qt_pool.tile([P, S], F32R, tag="qT", name=f"qT{i}") for i in range(3)]
        kT = [kt_pool.tile([P, S], F32R, tag="kT", name=f"kT{i}") for i in range(3)]
        v_tiles = [
            v_pool.tile([P, NH, VW], F16, tag="v", name=f"v{i}") for i in range(8)
        ]

        # rel stream pools first: their SBUF is disjoint from phase-1 pools,
        # so rel DMA + DVE adds run from t=0 and deep fp16 buffering keeps
        # the DMA queues fed across head boundaries.
        r1_pool = ctx.enter_context(tc.tile_pool(name="r1", bufs=2))
        r2_pool = ctx.enter_context(tc.tile_pool(name="r2", bufs=2))
        rbf_pool = ctx.enter_context(tc.tile_pool(name="rbf", bufs=16))

        # ---------------- phase 1: load, cast, transpose, project ----------
        with contextlib.ExitStack() as ph1:
            xload = ph1.enter_context(tc.tile_pool(name="xload", bufs=2))
            wload = ph1.enter_context(tc.tile_pool(name="wload", bufs=2))
            x16_pool = ph1.enter_context(tc.tile_pool(name="x16", bufs=8))
            w16_pool = ph1.enter_context(tc.tile_pool(name="w16", bufs=4))
            xt_pool = ph1.enter_context(tc.tile_pool(name="xT", bufs=6))
            wt_pool = ph1.enter_context(tc.tile_pool(name="wT", bufs=18))
            psum1 = ph1.enter_context(tc.tile_pool(name="psum1", bufs=3, space="PSUM"))
            psum1b = ph1.enter_context(
                tc.tile_pool(name="psum1b", bufs=2, space="PSUM")
            )

            # X tiles [128, 768] -> fp16
            x16 = []
            for t in range(8):
                xt_ = xload.tile([P, HIN], F32, tag="x")
                nc.sync.dma_start(xt_[:], x_ap[t * P:(t + 1) * P, :])
                x16_t = x16_pool.tile([P, HIN], F16, tag="x16", name=f"x16_{t}")
                nc.vector.tensor_copy(x16_t[:], xt_[:])
                x16.append(x16_t)

            # X^T: 6 fp16 tiles [128, 1024] (h-chunk on partitions)
            xT = []
            for hc in range(6):
                pt = psum1.tile([P, S], F16, tag="xtp")  # 1 bank (fp16)
                for t in range(8):
                    nc.tensor.transpose(
                        pt[:, t * P:(t + 1) * P],
                        x16[t][:, hc * P:(hc + 1) * P],
                        ident[:],
                    )
                xt_t = xt_pool.tile([P, S], F16, tag="xT")
                nc.scalar.copy(xt_t[:], pt[:])
                xT.append(xt_t)

            # W^T slices (fp16): wT[(w, hc)] = [128, 384]
            wT = {}
            for wname in ("q", "k", "v"):
                w_ap = aps[f"w{wname}"]
                w16s = []
                for d in range(3):
                    wt_ = wload.tile([P, HIN], F32, tag="wload")
                    nc.sync.dma_start(wt_[:], w_ap[d * P:(d + 1) * P, :])
                    w16_t = w16_pool.tile(
                        [P, HIN], F16, tag="w16", name=f"w16{wname}_{d}"
                    )
                    nc.vector.tensor_copy(w16_t[:], wt_[:])
                    w16s.append(w16_t)
                for hc in range(6):
                    pw = psum1b.tile([P, 512], F16, tag="ps1b", name="pw")[:, :HOUT]
                    for d in range(3):
                        nc.tensor.transpose(
                            pw[:, d * P:(d + 1) * P],
                            w16s[d][:, hc * P:(hc + 1) * P],
                            ident[:],
                        )
                    wt_t = wt_pool.tile([P, HOUT], F16, tag="wT")
                    nc.scalar.copy(wt_t[:], pw[:])
                    wT[(wname, hc)] = wt_t

            # Q^T, K^T projections: fp16 matmuls, fp32 PSUM, fp32r output
            for wname, dest, scale in (("q", qT, 0.125), ("k", kT, 1.0)):
                for d in range(3):
                    for tch in range(2):
                        pp = psum1b.tile([P, 512], F32, tag="projp")
                        for hc in range(6):
                            nc.tensor.matmul(
                                pp[:],
                                wT[(wname, hc)][:, d * P:(d + 1) * P],
                                xT[hc][:, tch * 512:(tch + 1) * 512],
                                start=(hc == 0),
                                stop=(hc == 5),
                            )
                        nc.scalar.activation(
                            dest[d][:, tch * 512:(tch + 1) * 512],
                            pp[:],
                            AF.Identity,
                            bias=bias_sb[wname][:, d:d + 1],
                            scale=scale,
                        )

            # V projection: out [t-tile 128, 384] fp16 + ones column
            for t in range(8):
                pv = psum1b.tile([P, 512], F32, tag="projp", name="pv")[:, :HOUT]
                for hc in range(6):
                    nc.tensor.matmul(
                        pv[:],
                        xT[hc][:, t * P:(t + 1) * P],
                        wT[("v", hc)][:],
                        start=(hc == 0),
                        stop=(hc == 5),
                    )
                nc.vector.memset(v_tiles[t][:], 1.0)
                # copy + bias add (bv broadcast along partitions)
                nc.vector.tensor_add(
                    v_tiles[t][:, :, 0:HD],
                    pv[:].rearrange("p (h d) -> p h d", d=HD),
                    bv_bc[:],
                )

        # ---------------- phase 2: attention per head ----------------
        out_pool = ctx.enter_context(tc.tile_pool(name="outst", bufs=8))
        out_stage = [
            out_pool.tile([P, HOUT], F32, tag="outst", name=f"outst{i}")
            for i in range(8)
        ]
        pt_pool = ctx.enter_context(tc.tile_pool(name="pT", bufs=12))
        fin_pool = ctx.enter_context(tc.tile_pool(name="fin", bufs=4))
        spsum = ctx.enter_context(tc.tile_pool(name="spsum", bufs=4, space="PSUM"))
        cpsum = ctx.enter_context(tc.tile_pool(name="cpsum", bufs=4, space="PSUM"))

        for h in range(NH):
            # rel12 = rel1 + rel2 -> fp16, four quarter tiles [128, 2, 1024]
            quarters = []
            for qq in range(4):
                r1 = r1_pool.tile([P, 2, S], F32, tag="r1")
                nc.sync.dma_start(
                    r1[:],
                    rel1_ap[h].rearrange("(qt p) k -> p qt k", p=P)[
                        :, qq * 2:(qq + 1) * 2, :
                    ],
                )
                r2 = r2_pool.tile([P, 2, S], F32, tag="r2")
                nc.sync.dma_start(
                    r2[:],
                    rel2_ap[h].rearrange("(qt p) k -> p qt k", p=P)[
                        :, qq * 2:(qq + 1) * 2, :
                    ],
                )
                rb = rbf_pool.tile([P, 2, S], F16, tag="rbf", name=f"rbf{h}_{qq}")
                nc.vector.tensor_add(rb[:], r1[:], r2[:])
                quarters.append(rb)

            dt, rem = divmod(h, 2)
            d0 = rem * HD
            qTh = qT[dt][d0:d0 + HD, :]
            kTh = kT[dt][d0:d0 + HD, :]

            ctx_ps = [
                cpsum.tile([P, 4 * VW], F32, tag="ctx", name=f"ctx{h}_{i}")
                for i in range(2)
            ]

            pT_strips = []
            for kt in range(KT):
                pT_strip = pt_pool.tile([P, S], F16, tag="pT", name=f"pT{h}_{kt}")
                pT_strips.append(pT_strip)
                for qch in range(2):
                    ps = spsum.tile([P, 512], F32, tag="sT")
                    # qk^T (fp32r: full-rate single-pass matmul)
                    nc.tensor.matmul(
                        ps[:],
                        kTh[:, kt * P:(kt + 1) * P],
                        qTh[:, qch * 512:(qch + 1) * 512],
                        start=True,
                        stop=False,
                    )
                    # += rel12^T (transposing adds via fp16 identity rhs)
                    for j in range(4):
                        qt = qch * 4 + j
                        nc.tensor.matmul(
                            ps[:, j * P:(j + 1) * P],
                            quarters[qt // 2][:, qt % 2, kt * P:(kt + 1) * P],
                            ident[:],
                            start=False,
                            stop=(j == 3),
                        )
                    # exp(scores + mask bias) -> fp16 probs
                    nc.scalar.activation(
                        pT_strip[:, qch * 512:(qch + 1) * 512],
                        ps[:],
                        AF.Exp,
                        bias=maskb[:, kt:kt + 1],
                        scale=1.0,
                    )

            # PV + denominator: per qt slot, contiguous accumulation group
            # (one open group per 2KB psum bank at a time)
            for qt in range(QT):
                cp = ctx_ps[qt // 4]
                sl = (qt % 4) * VW
                for kt in range(KT):
                    nc.tensor.matmul(
                        cp[:, sl:sl + VW],
                        pT_strips[kt][:, qt * P:(qt + 1) * P],
                        v_tiles[kt][:, h, :],
                        start=(kt == 0),
                        stop=(kt == KT - 1),
                    )

            # finalize: divide by denominator
            for qt in range(QT):
                cp = ctx_ps[qt // 4]
                sl = (qt % 4) * VW
                rc = fin_pool.tile([P, 1], F32, tag="recip")
                nc.vector.reciprocal(rc[:], cp[:, sl + HD:sl + HD + 1])
                nc.vector.tensor_scalar_mul(
                    out_stage[qt][:, h * HD:(h + 1) * HD],
                    cp[:, sl:sl + HD],
                    rc[:],
                )

        for qt in range(QT):
            nc.sync.dma_start(out_ap[qt * P:(qt + 1) * P, :], out_stage[qt][:])


def build_program():
    """Build and compile the per-core Bass program. Returns nc."""
    nc = bacc.Bacc(
        "TRN2",
        target_bir_lowering=False,
        debug=False,
        num_devices=8,
    )
    aps = {
        "x": nc.dram_tensor("x", [S, HIN], F32, kind="ExternalInput").ap(),
        "mask": nc.dram_tensor("mask", [S], I32, kind="ExternalInput").ap(),
        "rel1": nc.dram_tensor("rel1", [NH, S, S], F32, kind="ExternalInput").ap(),
        "rel2": nc.dram_tensor("rel2", [NH, S, S], F32, kind="ExternalInput").ap(),
        "wq": nc.dram_tensor("wq", [HOUT, HIN], F32, kind="ExternalInput").ap(),
        "wk": nc.dram_tensor("wk", [HOUT, HIN], F32, kind="ExternalInput").ap(),
        "wv": nc.dram_tensor("wv", [HOUT, HIN], F32, kind="ExternalInput").ap(),
        "bq": nc.dram_tensor("bq", [HOUT], F32, kind="ExternalInput").ap(),
        "bk": nc.dram_tensor("bk", [HOUT], F32, kind="ExternalInput").ap(),
        "bv": nc.dram_tensor("bv", [HOUT], F32, kind="ExternalInput").ap(),
        "out": nc.dram_tensor("out", [S, HOUT], F32, kind="ExternalOutput").ap(),
    }
    with tile.TileContext(nc) as tc:
        _build_kernel_body(tc, aps)
    nc.compile()
    return nc


def make_in_maps(inputs):
    """Slice full inputs into the 8 per-core input maps."""
    hs = np.ascontiguousarray(np.asarray(inputs["hidden_states"], np.float32))
    am = np.asarray(inputs["attention_mask"]).astype(np.int32)
    rel1 = np.asarray(inputs["rel_pos"], np.float32)
    rel2 = np.asarray(inputs["rel_2d_pos"], np.float32)
    ws = {k: np.asarray(inputs["W" + k[-1]], np.float32) for k in ("wq", "wk", "wv")}
    bs = {k: np.asarray(inputs["b" + k[-1]], np.float32) for k in ("bq", "bk", "bv")}

    in_maps = []
    for c in range(8):
        b, hh = divmod(c, 2)
        hsl = slice(hh * NH, (hh + 1) * NH)
        csl = slice(hh * HOUT, (hh + 1) * HOUT)
        m = {
            "x": np.ascontiguousarray(hs[b]),
            "mask": np.ascontiguousarray(am[b, 0, 0]),
            "rel1": np.ascontiguousarray(rel1[b, hsl]),
            "rel2": np.ascontiguousarray(rel2[b, hsl]),
        }
        for k in ("wq", "wk", "wv"):
            m[k] = np.ascontiguousarray(ws[k][csl])
        for k in ("bq", "bk", "bv"):
            m[k] = np.ascontiguousarray(bs[k][csl])
        in_maps.append(m)
    return in_maps


def gather_output(results):
    out = np.empty((4, S, HIN), np.float32)
    for c in range(8):
        b, hh = divmod(c, 2)
        out[b, :, hh * HOUT:(hh + 1) * HOUT] = results[c]["out"]
    return out


_NC_CACHE = []


def kernel(**inputs):
    if not _NC_CACHE:
        _NC_CACHE.append(build_program())
    nc = _NC_CACHE[0]
    in_maps = make_in_maps(inputs)
    res = run_bass_kernel_spmd(nc, in_maps, list(range(8)))
    return gather_output(res.results)


# revision 20
# speedup vs baseline: 1.1480x; 1.0153x over previous
"""ErnieLayout self-attention on 8 Trainium2 NeuronCores (Bass/Tile).

Problem shapes (hardcoded): B=4, S=1024, H=768, NH=12, HD=64.
Sharding: core c -> (batch b = c//2, head-half hh = c%2, i.e. 6 heads).
Each core computes attention for its 6 heads of one batch element and
writes the [S, 384] column slice of that batch's output.

Per-core algorithm (v3, mixed precision):
  setup:  X and W cast to fp16 (DVE), transposed on the PE (fp16 path);
          Q^T = (Wq_s @ X^T + bq)/8, K^T = Wk_s @ X^T + bk   (fp16 matmuls,
          fp32 PSUM accumulate, fp32r output tiles)
          V = X @ Wv_s^T (+ bv via DVE broadcast add), stored fp16 with a
          ones column appended (col 64 -> softmax denominator for free)
  per (head, ktile, q-chunk):
          psum[k=128, q=512] = K^T.T @ Q^T              (fp32r, 1 cyc/row)
          psum += rel12[q,ktile]^T via matmul(lhsT=rel12_f16, rhs=I_f16)
          pT = exp(psum + maskbias[k]) -> fp16          (ACT per-partition bias)
  per (head, qt): ctx[q,65] += pT.T @ V_aug[kt] over kt (fp16 matmuls)
  finalize: out[q, h*64:..] = ctx[:, :64] * (1/ctx[:, 64])  (fp32)

rel12 = rel_pos + rel_2d_pos is one DVE pass (fp32 in, fp16 out), streamed
in quarter-head strips so DMA prefetch runs continuously from t=0.
Masked keys get bias FLT_MIN so exp underflows to exactly 0 (matches the
reference's FLT_MIN replacement; no row-max needed, scores are O(10)).
Precision: fp16 carries >=10 mantissa bits -> final rel err ~1e-4..1e-3.
"""

import os
import sys

import numpy as np

for _p in ("/opt/trn_rl_repo",):
    if _p not in sys.path and os.path.isdir(_p):
        sys.path.append(_p)

import concourse.bass as bass
import concourse.mybir as mybir
import concourse.tile as tile
from concourse import bacc
from concourse.bass_utils import run_bass_kernel_spmd
from concourse.masks import make_identity

F32 = mybir.dt.float32
F32R = mybir.dt.float32r
F16 = mybir.dt.float16
I32 = mybir.dt.int32
AF = mybir.ActivationFunctionType
NEG = float(np.finfo(np.float32).min)

P = 128
S = 1024
NH = 6        # heads per core
HD = 64
HIN = 768     # model dim (contraction for projections)
HOUT = NH * HD  # 384, per-core projection width
KT = S // P   # 8 key tiles
QT = S // P   # 8 query tiles
VW = HD + 1   # 65: V columns + ones column


def _build_kernel_body(tc, aps):
    import contextlib

    nc = tc.nc
    x_ap = aps["x"]
    mask_ap = aps["mask"]
    rel1_ap = aps["rel1"]
    rel2_ap = aps["rel2"]
    out_ap = aps["out"]

    with contextlib.ExitStack() as ctx:
        const = ctx.enter_context(tc.tile_pool(name="const", bufs=1))

        ident = const.tile([P, P], F16)
        make_identity(nc, ident)
        ident32 = const.tile([P, P], F32)
        nc.vector.tensor_copy(ident32[:], ident[:])

        # mask bias: maskb[p, kt] = FLT_MIN if key (kt*128+p) masked else 0
        mask_i = const.tile([P, KT], I32)
        nc.sync.dma_start(mask_i[:], mask_ap.rearrange("(a p) -> p a", p=P))
        maskb = const.tile([P, KT], F32)
        nc.vector.tensor_copy(maskb[:], mask_i[:])
        nc.vector.tensor_scalar_mul(maskb[:], maskb[:], NEG)

        # biases: [384] -> [128, 3] (per-partition columns per d-tile)
        bias_sb = {}
        for wname in ("q", "k"):
            bt = const.tile([P, 3], F32, tag=f"b{wname}")
            nc.sync.dma_start(bt[:], aps[f"b{wname}"].rearrange("(a p) -> p a", p=P))
            if wname == "q":
                nc.vector.tensor_scalar_mul(bt[:], bt[:], 0.125)
            bias_sb[wname] = bt
        bv_bc = const.tile([P, NH, HD], F32)
        nc.sync.dma_start(
            bv_bc[:],
            aps["bv"].rearrange("(h d) -> h d", d=HD)[None].to_broadcast((P, NH, HD)),
        )

        # long-lived tensors
        qt_pool = ctx.enter_context(tc.tile_pool(name="qT", bufs=3))
        kt_pool = ctx.enter_context(tc.tile_pool(name="kT", bufs=3))
        v_pool = ctx.enter_context(tc.tile_pool(name="v", bufs=8))

        qT = [qt_pool.tile([P, S], F16, tag="qT", name=f"qT{i}") for i in range(3)]
        kT = [kt_pool.tile([P, S], F16, tag="kT", name=f"kT{i}") for i in range(3)]
        v_tiles = [
            v_pool.tile([P, NH, VW], F16, tag="v", name=f"v{i}") for i in range(8)
        ]

        # rel stream pools first: their SBUF is disjoint from phase-1 pools,
        # so rel DMA + DVE adds run from t=0 and deep fp16 buffering keeps
        # the DMA queues fed across head boundaries.
        r1_pool = ctx.enter_context(tc.tile_pool(name="r1", bufs=2))
        r2_pool = ctx.enter_context(tc.tile_pool(name="r2", bufs=2))
        rbf_pool = ctx.enter_context(tc.tile_pool(name="rbf", bufs=16))

        # ---------------- phase 1: load, cast, transpose, project ----------
        with contextlib.ExitStack() as ph1:
            xload = ph1.enter_context(tc.tile_pool(name="xload", bufs=2))
            wload = ph1.enter_context(tc.tile_pool(name="wload", bufs=2))
            x16_pool = ph1.enter_context(tc.tile_pool(name="x16", bufs=8))
            w16_pool = ph1.enter_context(tc.tile_pool(name="w16", bufs=4))
            xt_pool = ph1.enter_context(tc.tile_pool(name="xT", bufs=6))
            wt_pool = ph1.enter_context(tc.tile_pool(name="wT", bufs=18))
            psum1 = ph1.enter_context(tc.tile_pool(name="psum1", bufs=3, space="PSUM"))
            psum1b = ph1.enter_context(
                tc.tile_pool(name="psum1b", bufs=2, space="PSUM")
            )

            # X tiles [128, 768] -> fp16
            x16 = []
            for t in range(8):
                xt_ = xload.tile([P, HIN], F32, tag="x")
                nc.sync.dma_start(xt_[:], x_ap[t * P:(t + 1) * P, :])
                x16_t = x16_pool.tile([P, HIN], F16, tag="x16", name=f"x16_{t}")
                nc.vector.tensor_copy(x16_t[:], xt_[:])
                x16.append(x16_t)

            # X^T: 6 fp16 tiles [128, 1024] (h-chunk on partitions)
            xT = []
            for hc in range(6):
                pt = psum1.tile([P, S], F16, tag="xtp")  # 1 bank (fp16)
                for t in range(8):
                    nc.tensor.transpose(
                        pt[:, t * P:(t + 1) * P],
                        x16[t][:, hc * P:(hc + 1) * P],
                        ident[:],
                    )
                xt_t = xt_pool.tile([P, S], F16, tag="xT")
                nc.scalar.copy(xt_t[:], pt[:])
                xT.append(xt_t)

            # W^T slices (fp16): wT[(w, hc)] = [128, 384]
            wT = {}
            for wname in ("q", "k", "v"):
                w_ap = aps[f"w{wname}"]
                w16s = []
                for d in range(3):
                    wt_ = wload.tile([P, HIN], F32, tag="wload")
                    nc.sync.dma_start(wt_[:], w_ap[d * P:(d + 1) * P, :])
                    w16_t = w16_pool.tile(
                        [P, HIN], F16, tag="w16", name=f"w16{wname}_{d}"
                    )
                    nc.vector.tensor_copy(w16_t[:], wt_[:])
                    w16s.append(w16_t)
                for hc in range(6):
                    pw = psum1b.tile([P, 512], F16, tag="ps1b", name="pw")[:, :HOUT]
                    for d in range(3):
                        nc.tensor.transpose(
                            pw[:, d * P:(d + 1) * P],
                            w16s[d][:, hc * P:(hc + 1) * P],
                            ident[:],
                        )
                    wt_t = wt_pool.tile([P, HOUT], F16, tag="wT")
                    nc.scalar.copy(wt_t[:], pw[:])
                    wT[(wname, hc)] = wt_t

            # Q^T, K^T projections: fp16 matmuls, fp32 PSUM, fp32r output
            for wname, dest, scale in (("q", qT, 0.125), ("k", kT, 1.0)):
                for d in range(3):
                    for tch in range(2):
                        pp = psum1b.tile([P, 512], F32, tag="projp")
                        for hc in range(6):
                            nc.tensor.matmul(
                                pp[:],
                                wT[(wname, hc)][:, d * P:(d + 1) * P],
                                xT[hc][:, tch * 512:(tch + 1) * 512],
                                start=(hc == 0),
                                stop=(hc == 5),
                            )
                        nc.scalar.activation(
                            dest[d][:, tch * 512:(tch + 1) * 512],
                            pp[:],
                            AF.Identity,
                            bias=bias_sb[wname][:, d:d + 1],
                            scale=scale,
                        )

            # V projection: out [t-tile 128, 384] fp16 + ones column
            for t in range(8):
                pv = psum1b.tile([P, 512], F32, tag="projp", name="pv")[:, :HOUT]
                for hc in range(6):
                    nc.tensor.matmul(
                        pv[:],
                        xT[hc][:, t * P:(t + 1) * P],
                        wT[("v", hc)][:],
                        start=(hc == 0),
                        stop=(hc == 5),
                    )
                nc.vector.memset(v_tiles[t][:], 1.0)
                # copy + bias add (bv broadcast along partitions)
                nc.vector.tensor_add(
                    v_tiles[t][:, :, 0:HD],
                    pv[:].rearrange("p (h d) -> p h d", d=HD),
                    bv_bc[:],
                )

        # ---------------- phase 2: attention per head ----------------
        out_pool = ctx.enter_context(tc.tile_pool(name="outst", bufs=8))
        out_stage = [
            out_pool.tile([P, HOUT], F32, tag="outst", name=f"outst{i}")
            for i in range(8)
        ]
        pt_pool = ctx.enter_context(tc.tile_pool(name="pT", bufs=12))
        fin_pool = ctx.enter_context(tc.tile_pool(name="fin", bufs=4))
        spsum = ctx.enter_context(tc.tile_pool(name="spsum", bufs=4, space="PSUM"))
        cpsum = ctx.enter_context(tc.tile_pool(name="cpsum", bufs=2, space="PSUM"))
        vpsum = ctx.enter_context(tc.tile_pool(name="vpsum", bufs=2, space="PSUM"))
        ctt_pool = ctx.enter_context(tc.tile_pool(name="ctt", bufs=2))

        for h in range(NH):
            # rel12 = rel1 + rel2 -> fp16, four quarter tiles [128, 2, 1024]
            quarters = []
            for qq in range(4):
                r1 = r1_pool.tile([P, 2, S], F32, tag="r1")
                nc.sync.dma_start(
                    r1[:],
                    rel1_ap[h].rearrange("(qt p) k -> p qt k", p=P)[
                        :, qq * 2:(qq + 1) * 2, :
                    ],
                )
                r2 = r2_pool.tile([P, 2, S], F32, tag="r2")
                nc.sync.dma_start(
                    r2[:],
                    rel2_ap[h].rearrange("(qt p) k -> p qt k", p=P)[
                        :, qq * 2:(qq + 1) * 2, :
                    ],
                )
                rb = rbf_pool.tile([P, 2, S], F16, tag="rbf", name=f"rbf{h}_{qq}")
                nc.vector.tensor_add(rb[:], r1[:], r2[:])
                quarters.append(rb)

            dt, rem = divmod(h, 2)
            d0 = rem * HD
            qTh = qT[dt][d0:d0 + HD, :]
            kTh = kT[dt][d0:d0 + HD, :]

            ctx_ps = [
                cpsum.tile([P, 4 * VW], F32, tag="ctx", name=f"ctx{h}_{i}")
                for i in range(2)
            ]

            pT_strips = []
            for kt in range(KT):
                pT_strip = pt_pool.tile([P, S], F16, tag="pT", name=f"pT{h}_{kt}")
                pT_strips.append(pT_strip)
                for qch in range(2):
                    ps = spsum.tile([P, 512], F32, tag="sT")
                    # qk^T (fp32r: full-rate single-pass matmul)
                    nc.tensor.matmul(
                        ps[:],
                        kTh[:, kt * P:(kt + 1) * P],
                        qTh[:, qch * 512:(qch + 1) * 512],
                        start=True,
                        stop=False,
                    )
                    # += rel12^T (transposing adds via fp16 identity rhs)
                    for j in range(4):
                        qt = qch * 4 + j
                        nc.tensor.matmul(
                            ps[:, j * P:(j + 1) * P],
                            quarters[qt // 2][:, qt % 2, kt * P:(kt + 1) * P],
                            ident[:],
                            start=False,
                            stop=(j == 3),
                        )
                    # exp(scores + mask bias) -> fp16 probs
                    nc.scalar.activation(
                        pT_strip[:, qch * 512:(qch + 1) * 512],
                        ps[:],
                        AF.Exp,
                        bias=maskb[:, kt:kt + 1],
                        scale=1.0,
                    )

            # PV flipped: ctx^T[d|1, q] = V_aug.T @ P^T, accumulated over kt.
            # lhsT = V_aug tile (65 cols), rhs = pT strip (N=512 fp16) --
            # 16 big matmuls per head instead of 64 small ones. Row 64 of
            # ctx^T is the softmax denominator (ones column of V_aug).
            ctxT_ps = [
                vpsum.tile([VW, 512], F32, tag="ctxT", name=f"ctxT{h}_{i}")
                for i in range(2)
            ]
            for qch in range(2):
                for kt in range(KT):
                    nc.tensor.matmul(
                        ctxT_ps[qch][:],
                        v_tiles[kt][:, h, :],
                        pT_strips[kt][:, qch * 512:(qch + 1) * 512],
                        start=(kt == 0),
                        stop=(kt == KT - 1),
                    )
            ctxT_sb = [None, None]
            for qch in range(2):
                t_ = ctt_pool.tile([VW, 512], F32, tag="ctxT_sb",
                                   name=f"ctxTs{h}_{qch}")
                nc.scalar.copy(t_[:], ctxT_ps[qch][:])
                ctxT_sb[qch] = t_

            # back-transpose ctx^T -> ctx [q, 65] (fp32 exact), then divide
            for qt in range(QT):
                cp = ctx_ps[qt // 4]
                sl = (qt % 4) * VW
                nc.tensor.transpose(
                    cp[:, sl:sl + VW],
                    ctxT_sb[qt // 4 % 2 if False else qt // 4][
                        :, (qt % 4) * P:(qt % 4 + 1) * P
                    ],
                    ident32[:VW, :VW],
                )
                rc = fin_pool.tile([P, 1], F32, tag="recip")
                nc.vector.reciprocal(rc[:], cp[:, sl + HD:sl + HD + 1])
                nc.vector.tensor_scalar_mul(
                    out_stage[qt][:, h * HD:(h + 1) * HD],
                    cp[:, sl:sl + HD],
                    rc[:],
                )

        for qt in range(QT):
            nc.sync.dma_start(out_ap[qt * P:(qt + 1) * P, :], out_stage[qt][:])


def build_program():
    """Build and compile the per-core Bass program. Returns nc."""
    nc = bacc.Bacc(
        "TRN2",
        target_bir_lowering=False,
        debug=False,
        num_devices=8,
    )
    aps = {
        "x": nc.dram_tensor("x", [S, HIN], F32, kind="ExternalInput").ap(),
        "mask": nc.dram_tensor("mask", [S], I32, kind="ExternalInput").ap(),
        "rel1": nc.dram_tensor("rel1", [NH, S, S], F32, kind="ExternalInput").ap(),
        "rel2": nc.dram_tensor("rel2", [NH, S, S], F32, kind="ExternalInput").ap(),
        "wq": nc.dram_tensor("wq", [HOUT, HIN], F32, kind="ExternalInput").ap(),
        "wk": nc.dram_tensor("wk", [HOUT, HIN], F32, kind="ExternalInput").ap(),
        "wv": nc.dram_tensor("wv", [HOUT, HIN], F32, kind="ExternalInput").ap(),
        "bq": nc.dram_tensor("bq", [HOUT], F32, kind="ExternalInput").ap(),
        "bk": nc.dram_tensor("bk", [HOUT], F32, kind="ExternalInput").ap(),
        "bv": nc.dram_tensor("bv", [HOUT], F32, kind="ExternalInput").ap(),
        "out": nc.dram_tensor("out", [S, HOUT], F32, kind="ExternalOutput").ap(),
    }
    with tile.TileContext(nc) as tc:
        _build_kernel_body(tc, aps)
    nc.compile()
    return nc


def make_in_maps(inputs):
    """Slice full inputs into the 8 per-core input maps."""
    hs = np.ascontiguousarray(np.asarray(inputs["hidden_states"], np.float32))
    am = np.asarray(inputs["attention_mask"]).astype(np.int32)
    rel1 = np.asarray(inputs["rel_pos"], np.float32)
    rel2 = np.asarray(inputs["rel_2d_pos"], np.float32)
    ws = {k: np.asarray(inputs["W" + k[-1]], np.float32) for k in ("wq", "wk", "wv")}
    bs = {k: np.asarray(inputs["b" + k[-1]], np.float32) for k in ("bq", "bk", "bv")}

    in_maps = []
    for c in range(8):
        b, hh = divmod(c, 2)
        hsl = slice(hh * NH, (hh + 1) * NH)
        csl = slice(hh * HOUT, (hh + 1) * HOUT)
        m = {
            "x": np.ascontiguousarray(hs[b]),
            "mask": np.ascontiguousarray(am[b, 0, 0]),
            "rel1": np.ascontiguousarray(rel1[b, hsl]),
            "rel2": np.ascontiguousarray(rel2[b, hsl]),
        }
        for k in ("wq", "wk", "wv"):
            m[k] = np.ascontiguousarray(ws[k][csl])
        for k in ("bq", "bk", "bv"):
            m[k] = np.ascontiguousarray(bs[k][csl])
        in_maps.append(m)
    return in_maps


def gather_output(results):
    out = np.empty((4, S, HIN), np.float32)
    for c in range(8):
        b, hh = divmod(c, 2)
        out[b, :, hh * HOUT:(hh + 1) * HOUT] = results[c]["out"]
    return out


_NC_CACHE = []


def kernel(**inputs):
    if not _NC_CACHE:
        _NC_CACHE.append(build_program())
    nc = _NC_CACHE[0]
    in_maps = make_in_maps(inputs)
    res = run_bass_kernel_spmd(nc, in_maps, list(range(8)))
    return gather_output(res.results)
